# revision 62
# baseline (speedup 1.0000x reference)
"""Trainium2 Bass kernel for nn_Block_5617817223712 (BiPixelMamba + MSFF block).

Sharding: 8 cores = (batch 2) x (scan direction 2) x (d_inner half 2).
Each core runs the full LN1/LN2 prologue on its (possibly reversed) batch
image, the Mamba branch for its direction and d_inner half (selective scan
via the DVE tensor_tensor_scan instruction, per-state-index tiles with the
exp(delta*A) computed on ACT with per-partition scale), a 4-way AllReduce of
the out_proj partials, and then the MSFF branch on its quarter of the rows.

The SPMD program is identical on all cores; per-core behavior (which batch,
direction reversal, d-half weight slices, row-quarter) is driven entirely by
per-core input data (sliced/reversed/permuted on the host).
"""
import sys

sys.path.insert(0, "/opt/trn_rl_repo")

import numpy as np

C = 96          # d_model channels
H = 64
W = 64
L = H * W       # 4096 pixels
DI = 192        # d_inner
DH = 96         # d_inner half per core
DS = 16         # d_state
DTR = 6         # dt_rank
HID = 192       # msff hidden
CH = 512        # free-dim chunk (one PSUM bank of fp32)
NCH = L // CH   # 8
ROWS_SL = 22    # stage-D slice rows (16 + 2*3 halo)
LSL = ROWS_SL * W   # 1408
LQ = 16 * W         # 1024 valid out pixels per core
EXTW = W + 6        # 70, W-padded row width for dwconv input
EPS1 = 1e-6
EPS2 = 1e-5

_COMPILED = {}
DEBUG = False


def _chunks(total, size):
    out = []
    off = 0
    while off < total:
        out.append((off, min(size, total - off)))
        off += size
    return out


def _build():
    import concourse.bass as bass
    import concourse.bacc as bacc
    import concourse.mybir as mybir
    from concourse.tile import TileContext

    F32 = mybir.dt.float32
    F32R = mybir.dt.float32r
    BF16 = mybir.dt.bfloat16
    I32 = mybir.dt.int32
    AF = mybir.ActivationFunctionType
    OP = mybir.AluOpType

    nc = bacc.Bacc("TRN2", target_bir_lowering=False, debug=False, num_devices=8)

    def din(name, shape, dt=F32):
        return nc.declare_dram_parameter(name, list(shape), dt, isOutput=False)

    # ---- inputs (per-core data prepared on host) ----
    xin_d = din("xin", [C, L])
    xfwd_d = din("xfwd", [C, L])
    wconv_d = din("wconv", [C, 4 * DI])
    winz_d = din("winz", [C, DH])
    convb_d = din("convb", [DH, 2])
    wx_d = din("wx", [DH, 160])
    sel16_d = din("sel16", [80, 128], mybir.dt.bfloat16)
    sel96_d = din("sel96", [C, 12 * 128])
    selred_d = din("selred", [128, 12 * C], mybir.dt.bfloat16)
    apack_d = din("apack", [128, 12])
    wdt_d = din("wdt", [DTR, DH])
    bdt_d = din("bdt", [DH, 1])
    dpar_d = din("dpar", [DH, 1])
    wout_d = din("wout", [DH, C])
    lnw_d = din("lnw", [2, C])
    lnwc_d = din("lnwc", [1, C])
    stat_d = din("statw", [C, 4])
    stat6_d = din("stat6", [C, 6])
    lnsc_d = din("lnsc", [128, 6])
    w12row_d = din("w12row", [1, C])
    wb4_d = din("wb4", [36, C])
    lnwp_d = din("lnwp", [34, C])
    dwdiag_d = din("dwdiag", [DH, 3 * 9 * 2 * DH], mybir.dt.bfloat16)
    mask_d = din("mask", [C, 1])
    maskb_d = din("maskb", [C, 1])
    g1_d = din("gam1", [C, 1])
    g2_d = din("gam2", [C, 1])
    mwin_d = din("mwin", [C, 3 * HID])
    mwout_d = din("mwout", [DH, 2 * C])
    dwcol_d = din("dwcol", [DH, 54])
    dmask2_d = din("dmask2", [2, LSL])
    dmask128_d = din("dmask128", [128, LSL // 128])
    qoff_d = din("qoff", [1, 1], I32)
    onesrow_d = din("onesrow", [1, L])

    out_d = nc.declare_dram_parameter("out", [C, LQ], F32, isOutput=True)
    dbg = {}
    if DEBUG:
        for nm, shape in (("x1", [C, L]), ("xn", [C, L]), ("xs0", [C, L]),
                          ("zs", [C, L]), ("dbl", [80, L]), ("delta", [C, L]),
                          ("y", [C, L]), ("yb", [C, L]), ("yo", [C, L]),
                          ("xme", [C, (H + 6) * W]), ("xsl", [C, LSL]),
                          ("xm", [C, LSL]), ("g0", [DH, LQ]), ("g1", [DH, LQ]),
                          ("hh0", [C, L]), ("dA0", [C, L]), ("dBx0", [C, L])):
            dbg[nm] = nc.declare_dram_parameter("dbg_" + nm, shape, F32,
                                                isOutput=True)

    yo_in_b = nc.dram_tensor("yo_in_b", [C, L], BF16)
    yo_out_b = nc.dram_tensor("yo_out_b", [C, L], BF16)

    SL_BYTES = (H + 6) * W  # largest slot free-elems (xme) = 4480

    with TileContext(nc) as tc:
        with tc.tile_pool(name="pw", bufs=1) as pw, \
             tc.tile_pool(name="pm", bufs=1) as pm, \
             tc.tile_pool(name="pt", bufs=2) as pt, \
             tc.tile_pool(name="pp", bufs=1, space="PSUM") as pp:

            def slot(name, shape, dt, tag):
                return pm.tile(list(shape), dt, name=name, tag=tag)

            # ---------- weights ----------
            def wtile(dram, shape, dt=F32):
                t = pw.tile(list(shape), dt, name="w_" + dram.name)
                s = dram[tuple(slice(None) for _ in shape)]
                if dt == F32R:
                    s = s.bitcast(F32R)
                nc.sync.dma_start(out=t, in_=s)
                return t

            # xin first so the LN stats pipeline starts before the ~4MB of
            # weight DMAs finish.
            xin_t = slot("xin", [C, L], F32R, "sl0")
            nc.sync.dma_start(out=xin_t, in_=xin_d[:, :].bitcast(F32R))

            wconv_t = wtile(wconv_d, [C, 4 * DI], F32R)
            winz_t = wtile(winz_d, [C, DH], F32R)
            convb_t = wtile(convb_d, [DH, 2])
            wx_t = wtile(wx_d, [DH, 160], F32R)
            sel16_t = wtile(sel16_d, [80, 128], BF16)
            sel96_t = wtile(sel96_d, [C, 12 * 128], F32R)
            selred_t = wtile(selred_d, [128, 12 * C], BF16)
            apack_t = wtile(apack_d, [128, 12])
            wdt_t = wtile(wdt_d, [DTR, DH], F32R)
            bdt_t = wtile(bdt_d, [DH, 1])
            dpar_t = wtile(dpar_d, [DH, 1])
            wout_t = wtile(wout_d, [DH, C], F32R)
            lnw_t = wtile(lnw_d, [2, C], F32R)
            lnwc_t = wtile(lnwc_d, [1, C], F32R)
            stat_t = wtile(stat_d, [C, 4], F32R)
            stat6_t = wtile(stat6_d, [C, 6], F32R)
            lnsc_t = wtile(lnsc_d, [128, 6])
            w12row_t = wtile(w12row_d, [1, C], F32R)
            wb4_t = wtile(wb4_d, [36, C], F32R)
            lnwp_t = wtile(lnwp_d, [34, C], F32R)
            mask_t = wtile(mask_d, [C, 1])
            maskb_t = wtile(maskb_d, [C, 1])
            g1_t = wtile(g1_d, [C, 1])
            g2_t = wtile(g2_d, [C, 1])
            mwin_t = wtile(mwin_d, [C, 3 * HID], F32R)
            mwout_t = wtile(mwout_d, [DH, 2 * C], F32R)
            dwcol_t = wtile(dwcol_d, [DH, 54])
            dmask2_t = wtile(dmask2_d, [2, LSL])
            dmask128_t = wtile(dmask128_d, [128, LSL // 128])
            qoff_t = wtile(qoff_d, [1, 1], I32)

            eps_t = pw.tile([128, 2], F32, name="eps_t")
            nc.vector.memset(eps_t[:, 0:1], EPS1)
            nc.vector.memset(eps_t[:, 1:2], EPS2)

            def rev(ap2d, Lx):
                return bass.AP(
                    tensor=ap2d.tensor,
                    offset=ap2d.offset + (Lx - 1),
                    ap=[list(ap2d.ap[0]), [-1, Lx]],
                )

            # ---------- LN helpers ----------
            def ln_stats(src_t, Lx, eps_col, rows_name):
                sq = slot("sq_" + rows_name, [C, Lx], F32R, "sl1")
                nc.scalar.activation(out=sq, in_=src_t[:, 0:Lx].bitcast(F32),
                                     func=AF.Square)
                S = slot("S_" + rows_name, [2, Lx], F32, "sl2")
                for off, sz in _chunks(Lx, CH):
                    ps = pp.tile([2, CH], F32, tag="bc1", bufs=2, name="lnps")
                    nc.tensor.matmul(out=ps[:, 0:sz], lhsT=stat_t[:, 0:2],
                                     rhs=src_t[:, off:off + sz],
                                     start=True, stop=False)
                    nc.tensor.matmul(out=ps[:, 0:sz], lhsT=stat_t[:, 2:4],
                                     rhs=sq[:, off:off + sz],
                                     start=False, stop=True)
                    nc.scalar.copy(out=S[:, off:off + sz], in_=ps[:, 0:sz])
                k = Lx // 128
                Sr = pt.tile([128, 2 * k], F32, tag="Sr", name="Sr")
                nc.sync.dma_start(
                    out=Sr[:, 0:k],
                    in_=S[0:1, :].rearrange("o (p k) -> o p k", k=k))
                nc.sync.dma_start(
                    out=Sr[:, k:2 * k],
                    in_=S[1:2, :].rearrange("o (p k) -> o p k", k=k))
                u = pt.tile([128, k], F32, tag="u_r", name="u_r")
                nc.vector.tensor_scalar_mul(u, Sr[:, 0:k], 1.0 / C)
                usq = pt.tile([128, k], F32, tag="usq_r", name="usq_r")
                nc.vector.tensor_tensor(out=usq, in0=u, in1=u, op=OP.mult)
                var = pt.tile([128, k], F32, tag="var_r", name="var_r")
                nc.vector.scalar_tensor_tensor(
                    out=var, in0=Sr[:, k:2 * k], scalar=1.0 / C, in1=usq,
                    op0=OP.mult, op1=OP.subtract)
                lnv = pt.tile([128, k], F32, tag="lnv_r", name="lnv_r")
                nc.scalar.activation(out=lnv, in_=var, func=AF.Ln,
                                     bias=eps_t[:, eps_col:eps_col + 1])
                rr = pt.tile([128, k], F32, tag="rr_r", name="rr_r")
                nc.scalar.activation(out=rr, in_=lnv, func=AF.Exp, scale=-0.5)
                ar = pt.tile([128, k], F32, tag="ar_r", name="ar_r")
                nc.vector.scalar_tensor_tensor(
                    out=ar, in0=u, scalar=-1.0, in1=rr, op0=OP.mult, op1=OP.mult)
                rows = slot(rows_name, [2, Lx], F32R, "sl3")
                nc.sync.dma_start(
                    out=rows[0:1, :].rearrange("o (p k) -> o p k", k=k),
                    in_=rr.bitcast(F32R))
                nc.sync.dma_start(
                    out=rows[1:2, :].rearrange("o (p k) -> o p k", k=k),
                    in_=ar.bitcast(F32R))
                return rows, rr, ar

            def ln_apply(src_t, r_src, ra_t, wc_t, wb_t, out_t, out_off, Lx):
                for off, sz in _chunks(Lx, CH):
                    rw = pp.tile([C, CH], F32, tag="bc1", bufs=2, name="rw")
                    nc.tensor.matmul(out=rw[:, 0:sz], lhsT=wc_t,
                                     rhs=r_src[0:1, off:off + sz],
                                     start=True, stop=True)
                    aw = pp.tile([C, CH], F32, tag="bc2", bufs=2, name="aw")
                    nc.tensor.matmul(out=aw[:, 0:sz], lhsT=wb_t,
                                     rhs=ra_t[:, off:off + sz],
                                     start=True, stop=True)
                    t = pt.tile([C, CH], F32, tag="lnt", name="lnt")
                    nc.vector.tensor_tensor(out=t[:, 0:sz],
                                            in0=src_t[:, off:off + sz].bitcast(F32),
                                            in1=rw[:, 0:sz], op=OP.mult)
                    nc.vector.tensor_tensor(
                        out=out_t[:, out_off + off:out_off + off + sz],
                        in0=t[:, 0:sz], in1=aw[:, 0:sz], op=OP.add)

            # ---------- stage A: fused double-LN ----------
            # Both LN1 (channels-first) and the inner LN (also over C) are
            # derived from a single stats pass over x: six weighted channel
            # sums {x, w1 x, w1^2 x, w1 b1 x} and {x^2, w1^2 x^2} give the
            # row stats of x1 = LN1(x) algebraically.
            sqa = slot("sqa", [C, L], F32R, "sl1")
            nc.scalar.activation(out=sqa, in_=xin_t.bitcast(F32),
                                 func=AF.Square)
            S6 = slot("S6", [34, L], F32, "sl2")
            for off, sz in _chunks(L, CH):
                ps1 = pp.tile([4, CH], F32, tag="bc1", bufs=2, name="st1")
                nc.tensor.matmul(out=ps1[:, 0:sz], lhsT=stat6_t[:, 0:4],
                                 rhs=xin_t[:, off:off + sz],
                                 start=True, stop=True)
                nc.scalar.copy(out=S6[0:4, off:off + sz], in_=ps1[:, 0:sz])
                ps2 = pp.tile([2, CH], F32, tag="bc2", bufs=2, name="st2")
                nc.tensor.matmul(out=ps2[:, 0:sz], lhsT=stat6_t[:, 4:6],
                                 rhs=sqa[:, off:off + sz],
                                 start=True, stop=True)
                nc.scalar.copy(out=S6[32:34, off:off + sz], in_=ps2[:, 0:sz])
            ka = L // 128
            R6 = pt.tile([128, 6 * ka], F32, tag="R6", name="R6", bufs=1)
            for i, sr in enumerate((0, 1, 2, 3, 32, 33)):
                nc.sync.dma_start(
                    out=R6[:, i * ka:(i + 1) * ka],
                    in_=S6[sr:sr + 1, :].rearrange("o (p k) -> o p k", k=ka))

            def row(name):
                return pt.tile([128, ka], F32, tag="row_" + name, name=name,
                               bufs=1)

            TT = nc.vector.tensor_tensor
            STT = nc.vector.scalar_tensor_tensor
            mu1 = row("mu1")
            nc.vector.tensor_scalar_mul(mu1, R6[:, 0:ka], 1.0 / C)
            usq1 = row("usq1")
            TT(out=usq1, in0=mu1, in1=mu1, op=OP.mult)
            var1 = row("var1")
            STT(out=var1, in0=R6[:, 4 * ka:5 * ka], scalar=1.0 / C, in1=usq1,
                op0=OP.mult, op1=OP.subtract)
            lnv1 = row("lnv1")
            nc.scalar.activation(out=lnv1, in_=var1, func=AF.Ln,
                                 bias=eps_t[:, 0:1])
            r1 = row("r1")
            nc.scalar.activation(out=r1, in_=lnv1, func=AF.Exp, scale=-0.5)
            r1mu1 = row("r1mu1")
            TT(out=r1mu1, in0=r1, in1=mu1, op=OP.mult)
            rA = row("rA")
            TT(out=rA, in0=r1, in1=R6[:, ka:2 * ka], op=OP.mult)
            S2r = row("S2r")
            STT(out=S2r, in0=r1mu1, scalar=lnsc_t[:, 0:1], in1=rA,
                op0=OP.mult, op1=OP.add)
            mu2 = row("mu2")
            nc.scalar.activation(out=mu2, in_=S2r, func=AF.Identity,
                                 scale=1.0 / C, bias=lnsc_t[:, 1:2])
            m2D = row("m2D")
            TT(out=m2D, in0=mu1, in1=R6[:, 2 * ka:3 * ka], op=OP.mult)
            T1a = row("T1a")
            STT(out=T1a, in0=m2D, scalar=-2.0, in1=R6[:, 5 * ka:6 * ka],
                op0=OP.mult, op1=OP.add)
            T1 = row("T1")
            STT(out=T1, in0=usq1, scalar=lnsc_t[:, 2:3], in1=T1a,
                op0=OP.mult, op1=OP.add)
            T2 = row("T2")
            STT(out=T2, in0=mu1, scalar=lnsc_t[:, 3:4], in1=R6[:, 3 * ka:4 * ka],
                op0=OP.mult, op1=OP.add)
            r1sq = row("r1sq")
            TT(out=r1sq, in0=r1, in1=r1, op=OP.mult)
            Q2a = row("Q2a")
            TT(out=Q2a, in0=r1sq, in1=T1, op=OP.mult)
            Q2b = row("Q2b")
            TT(out=Q2b, in0=r1, in1=T2, op=OP.mult)
            Q2 = row("Q2")
            STT(out=Q2, in0=Q2b, scalar=2.0, in1=Q2a, op0=OP.mult, op1=OP.add)
            q2c = row("q2c")
            nc.scalar.activation(out=q2c, in_=Q2, func=AF.Identity,
                                 scale=1.0 / C, bias=lnsc_t[:, 4:5])
            usq2 = row("usq2")
            TT(out=usq2, in0=mu2, in1=mu2, op=OP.mult)
            var2 = row("var2")
            TT(out=var2, in0=q2c, in1=usq2, op=OP.subtract)
            lnv2 = row("lnv2")
            nc.scalar.activation(out=lnv2, in_=var2, func=AF.Ln,
                                 bias=eps_t[:, 1:2])
            r2 = row("r2")
            nc.scalar.activation(out=r2, in_=lnv2, func=AF.Exp, scale=-0.5)
            Pr = row("Pr")
            TT(out=Pr, in0=r1, in1=r2, op=OP.mult)
            Pmu1 = row("Pmu1")
            TT(out=Pmu1, in0=Pr, in1=mu1, op=OP.mult)
            r2mu2 = row("r2mu2")
            TT(out=r2mu2, in0=r2, in1=mu2, op=OP.mult)
            ar1 = row("ar1")
            STT(out=ar1, in0=mu1, scalar=-1.0, in1=r1, op0=OP.mult, op1=OP.mult)

            rxn = slot("rxn", [36, L], F32R, "sl3")
            for i, rt in ((0, Pr), (32, Pmu1), (33, r2), (34, r2mu2)):
                nc.sync.dma_start(
                    out=rxn[i:i + 1, :].rearrange("o (p k) -> o p k", k=ka),
                    in_=rt.bitcast(F32R))
            nc.sync.dma_start(out=rxn[35:36, :],
                              in_=onesrow_d[:, 0:L].bitcast(F32R))
            rx1 = slot("rx1", [34, L], F32R, "sl4")
            for i, rt in ((0, r1), (33, ar1)):
                nc.sync.dma_start(
                    out=rx1[i:i + 1, :].rearrange("o (p k) -> o p k", k=ka),
                    in_=rt.bitcast(F32R))
            nc.sync.dma_start(out=rx1[32:33, :],
                              in_=onesrow_d[:, 0:L].bitcast(F32R))

            xn_t = slot("xn", [C, L + 3], F32R, "sl6")
            nc.vector.memset(xn_t[:, 0:3].bitcast(F32), 0.0)
            ln_apply(xin_t, rxn[0:1, :], rxn[32:36, :], w12row_t,
                     wb4_t[32:36, :], xn_t, 3, L)
            x1_t = slot("x1", [C, L], F32R, "sl5")
            ln_apply(xin_t, rx1[0:1, :], rx1[32:34, :], lnwc_t,
                     lnwp_t[32:34, :], x1_t, 0, L)

            # ---------- stage B ----------
            xs0_t = slot("xs0", [C, L], F32R, "sl0")
            xs1_t = slot("xs1", [C, L], F32R, "sl1")
            zs_t = slot("zs", [C, L], BF16, "sl2")
            for off, sz in _chunks(L, CH):
                for mt, xs_t in ((0, xs0_t), (1, xs1_t)):
                    ps = pp.tile([C, CH], F32, tag="bc1", bufs=2, name="cps")
                    for t in range(4):
                        nc.tensor.matmul(
                            out=ps[:, 0:sz],
                            lhsT=wconv_t[:, t * DI + mt * DH:t * DI + (mt + 1) * DH],
                            rhs=xn_t[:, off + t:off + t + sz],
                            start=(t == 0), stop=(t == 3))
                    nc.scalar.activation(out=xs_t[:, off:off + sz], in_=ps[:, 0:sz],
                                         func=AF.Silu, bias=convb_t[:, mt:mt + 1])
                psz = pp.tile([C, CH], F32, tag="bc2", bufs=2, name="zps")
                nc.tensor.matmul(out=psz[:, 0:sz], lhsT=winz_t,
                                 rhs=xn_t[:, 3 + off:3 + off + sz],
                                 start=True, stop=True)
                nc.scalar.activation(out=zs_t[:, off:off + sz], in_=psz[:, 0:sz],
                                     func=AF.Silu)

            dbl_t = slot("dbl", [80, L], F32R, "sl3")
            dblbf_t = pw.tile([80, L], BF16, name="dblbf")
            for off, sz in _chunks(L, CH):
                ps = pp.tile([80, CH], F32, tag="bc1", bufs=2, name="dps")
                nc.tensor.matmul(out=ps[:, 0:sz], lhsT=wx_t[:, 0:80],
                                 rhs=xs0_t[:, off:off + sz], start=True, stop=False)
                nc.tensor.matmul(out=ps[:, 0:sz], lhsT=wx_t[:, 80:160],
                                 rhs=xs1_t[:, off:off + sz], start=False, stop=True)
                nc.scalar.copy(out=dbl_t[:, off:off + sz], in_=ps[:, 0:sz])
                nc.vector.tensor_copy(out=dblbf_t[32:64, off:off + sz],
                                      in_=ps[32:64, 0:sz])
                nc.vector.tensor_copy(out=dblbf_t[64:80, off:off + sz],
                                      in_=ps[64:80, 0:sz])

            esp_t = slot("esp", [C, L], F32, "sl4")
            for off, sz in _chunks(L, CH):
                ps = pp.tile([C, CH], F32, tag="bc1", bufs=2, name="dtps")
                nc.tensor.matmul(out=ps[:, 0:sz], lhsT=wdt_t,
                                 rhs=dbl_t[0:DTR, off:off + sz],
                                 start=True, stop=True)
                nc.scalar.activation(out=esp_t[:, off:off + sz], in_=ps[:, 0:sz],
                                     func=AF.Exp, bias=bdt_t[:, 0:1])
            delta_t = slot("delta", [C, L], F32R, "sl6")
            nc.scalar.activation(out=delta_t, in_=esp_t, func=AF.Ln, bias=1.0)

            u_t = slot("u", [C, L], F32R, "sl1")
            nc.vector.tensor_tensor(out=u_t, in0=delta_t.bitcast(F32),
                                    in1=xs0_t.bitcast(F32), op=OP.mult)

            # ---------- packed selective scan ----------
            # (d, s) pairs packed onto 128 partitions: p = 16*dl + s, 12
            # chunks j covering d = 8j + dl. Broadcasts of delta/u into the
            # packed layout via 0/1-selector matmuls; dA = exp on Scalar with
            # per-partition A scale; scan on DVE; w = hh*C on gpsimd; the
            # sum over s via selector matmuls into resident PSUM banks.
            LH = L // 2
            NCHK = 12
            bbcc_t = slot("bbcc", [128, 2 * L], BF16, "sl3")
            for off, sz in _chunks(L, CH):
                pb = pp.tile([128, CH], F32, tag="bc1", bufs=2, name="pbb")
                nc.tensor.matmul(out=pb[:, 0:sz], lhsT=sel16_t[32:48, :],
                                 rhs=dblbf_t[32:48, off:off + sz],
                                 start=True, stop=True)
                nc.scalar.copy(out=bbcc_t[:, off:off + sz], in_=pb[:, 0:sz])
                pc = pp.tile([128, CH], F32, tag="bc2", bufs=2, name="pcc")
                nc.tensor.matmul(out=pc[:, 0:sz], lhsT=sel16_t[64:80, :],
                                 rhs=dblbf_t[64:80, off:off + sz],
                                 start=True, stop=True)
                nc.scalar.copy(out=bbcc_t[:, L + off:L + off + sz],
                               in_=pc[:, 0:sz])

            carry_t = pw.tile([128, NCHK], F32, name="carry_t")
            for hfl in range(2):
                ho = hfl * LH
                yh = slot(f"y{hfl}", [C, LH], F32R, "sl4")
                yps = [pp.tile([C, CH], F32, tag=f"yp{q}", bufs=1,
                               name=f"yps{hfl}{q}") for q in range(4)]
                for j in range(NCHK):
                    dA = slot(f"dA{hfl}_{j}", [128, LH], BF16,
                              "sl7" if j % 2 == 0 else "sl8")
                    dBx = slot(f"dBx{hfl}_{j}", [128, LH], BF16,
                               "sl9" if j % 2 == 0 else "slA")
                    for q in range(4):
                        co = ho + q * CH
                        psD = pp.tile([128, CH], F32, tag="bc1", bufs=2,
                                      name="psD")
                        nc.tensor.matmul(out=psD,
                                         lhsT=sel96_t[:, j * 128:(j + 1) * 128],
                                         rhs=delta_t[:, co:co + CH],
                                         start=True, stop=True)
                        nc.scalar.activation(out=dA[:, q * CH:(q + 1) * CH],
                                             in_=psD, func=AF.Exp,
                                             scale=apack_t[:, j:j + 1])
                        psU = pp.tile([128, CH], F32, tag="bc2", bufs=2,
                                      name="psU")
                        nc.tensor.matmul(out=psU,
                                         lhsT=sel96_t[:, j * 128:(j + 1) * 128],
                                         rhs=u_t[:, co:co + CH],
                                         start=True, stop=True)
                        nc.vector.tensor_tensor(out=dBx[:, q * CH:(q + 1) * CH],
                                                in0=psU,
                                                in1=bbcc_t[:, co:co + CH],
                                                op=OP.mult)
                    # reduce of the previous chunk goes to the PE queue after
                    # this chunk's broadcasts so chunk j+1 is never blocked
                    # behind w(j) on the in-order PE queue.
                    if j >= 1:
                        hhp = hh
                        for q in range(4):
                            nc.tensor.matmul(
                                out=yps[q],
                                lhsT=selred_t[:, (j - 1) * C:j * C],
                                rhs=hhp[:, q * CH:(q + 1) * CH],
                                start=(j == 1), stop=False)
                    hh = slot(f"hh{hfl}_{j}", [128, LH], BF16,
                              "slB" if j % 2 == 0 else "slC")
                    nc.vector.tensor_tensor_scan(
                        out=hh, data0=dA, data1=dBx,
                        initial=(0.0 if hfl == 0 else carry_t[:, j:j + 1]),
                        op0=OP.mult, op1=OP.add)
                    if hfl == 0:
                        nc.scalar.copy(out=carry_t[:, j:j + 1],
                                       in_=hh[:, LH - 1:LH])
                    nc.gpsimd.tensor_tensor(out=hh, in0=hh,
                                            in1=bbcc_t[:, L + ho:L + ho + LH],
                                            op=OP.mult)
                for q in range(4):
                    nc.tensor.matmul(out=yps[q],
                                     lhsT=selred_t[:, (NCHK - 1) * C:NCHK * C],
                                     rhs=hh[:, q * CH:(q + 1) * CH],
                                     start=False, stop=True)
                for q in range(4):
                    nc.vector.scalar_tensor_tensor(
                        out=yh[:, q * CH:(q + 1) * CH],
                        in0=xs0_t[:, ho + q * CH:ho + (q + 1) * CH].bitcast(F32),
                        scalar=dpar_t[:, 0:1], in1=yps[q],
                        op0=OP.mult, op1=OP.add)

                # gate + blend within this compute half. For dir=1 cores the
                # within-half reversal of compute-half hfl equals forward cols
                # of the OTHER half, so each core writes its content into the
                # appropriate bounce half (other half zeroed; AllReduce adds).
                nc.vector.tensor_tensor(out=yh, in0=yh.bitcast(F32),
                                        in1=zs_t[:, ho:ho + LH], op=OP.mult)
                # out_proj on the un-blended half; the reversal for the
                # backward direction commutes with the channel matmul, so it
                # moves into the zmb DMA source stride instead of DVE blends.
                zm = slot(f"zm{hfl}", [C, LH], BF16, "sl8")
                zmb = slot(f"zmb{hfl}", [C, LH], BF16, "slZ")
                for off, sz in _chunks(LH, CH):
                    ps = pp.tile([C, CH], F32, tag="bc1", bufs=2, name="ops")
                    nc.tensor.matmul(out=ps[:, 0:sz], lhsT=wout_t,
                                     rhs=yh[:, off:off + sz],
                                     start=True, stop=True)
                    nc.scalar.activation(out=zm[:, off:off + sz],
                                         in_=ps[:, 0:sz], func=AF.Copy,
                                         scale=mask_t[:, 0:1])
                    nc.scalar.activation(out=zmb[:, off:off + sz],
                                         in_=ps[:, 0:sz], func=AF.Copy,
                                         scale=maskb_t[:, 0:1])
                nc.sync.dma_start(out=yo_in_b[:, ho:ho + LH], in_=zm[:, :])
                nc.sync.dma_start(out=yo_in_b[:, L - LH - ho:L - ho],
                                  in_=rev(zmb[:, :], LH))

            # ---------- stage C prep (independent of yo: overlaps the
            # AllReduce on the in-order engine queues) ----------
            t2 = slot("t2", [C, L], F32, "sl0")
            nc.vector.tensor_scalar_mul(t2, rev(x1_t[:, :].bitcast(F32), L),
                                        maskb_t[:, 0:1])
            x1c_t = slot("x1c", [C, L], F32, "sl1")
            nc.vector.scalar_tensor_tensor(out=x1c_t, in0=x1_t.bitcast(F32),
                                           scalar=mask_t[:, 0:1],
                                           in1=t2, op0=OP.mult, op1=OP.add)
            xfwd_t = slot("xfwd", [C, L], F32, "sl2")
            nc.sync.dma_start(out=xfwd_t, in_=xfwd_d[:, :])
            q_t = slot("q", [C, L], F32, "sl3")
            nc.vector.scalar_tensor_tensor(out=q_t, in0=x1c_t,
                                           scalar=g1_t[:, 0:1],
                                           in1=xfwd_t, op0=OP.mult, op1=OP.add)
            xme_t = slot("xme", [C, (H + 6) * W], F32, "sl6")
            nc.gpsimd.memset(xme_t[:, 0:3 * W], 0.0)
            nc.gpsimd.memset(xme_t[:, (H + 3) * W:(H + 6) * W], 0.0)

            nc.gpsimd.collective_compute(
                "AllReduce", OP.add,
                replica_groups=[[0, 1, 2, 3], [4, 5, 6, 7]],
                ins=[yo_in_b.ap().opt()],
                outs=[yo_out_b.ap().opt()],
            )
            yo_t = slot("yo", [C, L], BF16, "sl9")
            nc.sync.dma_start(out=yo_t, in_=yo_out_b[:, :])

            if DEBUG:
                nc.sync.dma_start(out=dbg["yo"][:, :], in_=yo_t[:, :])
            # ---------- stage C ----------
            nc.vector.scalar_tensor_tensor(
                out=xme_t[:, 3 * W:(H + 3) * W], in0=yo_t, scalar=g1_t[:, 0:1],
                in1=q_t, op0=OP.mult, op1=OP.add)

            # ---------- stage D ----------
            reg = nc.alloc_registers()
            nc.regs_load(reg, qoff_t[0:1, 0:1])
            qoff = nc.snap(reg, min_val=0, max_val=3 * 1024)
            xsl_t = slot("xsl", [C, LSL], F32R, "sl0")
            nc.scalar.copy(out=xsl_t, in_=xme_t[:, bass.ds(qoff, LSL)])

            if DEBUG:
                nc.sync.dma_start(out=dbg["xme"][:, :], in_=xme_t[:, :])
                nc.sync.dma_start(out=dbg["xsl"][:, :], in_=xsl_t.bitcast(F32))
            rows3, rr3, ar3 = ln_stats(xsl_t, LSL, 0, "rows3")
            k3 = LSL // 128
            rmr = pt.tile([128, k3], F32, tag="rmr", name="rmr")
            nc.vector.tensor_tensor(out=rmr, in0=rr3, in1=dmask128_t, op=OP.mult)
            amr = pt.tile([128, k3], F32, tag="amr", name="amr")
            nc.vector.tensor_tensor(out=amr, in0=ar3, in1=dmask128_t, op=OP.mult)
            rm = slot("rm", [1, LSL], F32R, "sl8")
            nc.sync.dma_start(
                out=rm[0:1, :].rearrange("o (p k) -> o p k", k=k3),
                in_=rmr.bitcast(F32R))
            ra3 = slot("ra3", [2, LSL], F32R, "sl4")
            nc.sync.dma_start(out=ra3[0:1, :],
                              in_=dmask2_t[0:1, :].bitcast(F32R))
            nc.sync.dma_start(
                out=ra3[1:2, :].rearrange("o (p k) -> o p k", k=k3),
                in_=amr.bitcast(F32R))
            xm_t = slot("xm", [C, LSL], F32R, "sl5")
            ln_apply(xsl_t, rm, ra3, lnwc_t, lnw_t, xm_t, 0, LSL)

            if DEBUG:
                nc.sync.dma_start(out=dbg["xm"][:, :], in_=xm_t.bitcast(F32))
            g_t = [slot(f"g{tt}", [DH, LQ], F32R, f"sl{2 + tt}")
                   for tt in range(2)]
            row_chunks = [(0, 8), (8, 8), (16, 6)]
            # depthwise 3x3 conv as 9 chained per-partition-scalar FMAs on
            # DVE/Pool (the weights are diagonal — no matmuls needed).
            for j in range(3):
                dil = j + 1
                dwj = pw.tile([DH, 9 * 2 * DH], BF16, tag="dwj", bufs=1,
                              name=f"dwj{j}")
                nc.sync.dma_start(
                    out=dwj,
                    in_=dwdiag_d[:, j * 9 * 2 * DH:(j + 1) * 9 * 2 * DH])
                for tt in range(2):
                    hp = slot(f"hp{j}{tt}", [DH, ROWS_SL * EXTW], BF16,
                              f"sl{7 + tt}")
                    nc.gpsimd.memset(hp[:, :], 0.0)
                    m0 = j * HID + tt * DH
                    for r0, nr in row_chunks:
                        ps = pp.tile([DH, CH], F32, tag="bc1", bufs=2, name="pips")
                        nc.tensor.matmul(
                            out=ps[:, 0:nr * W], lhsT=mwin_t[:, m0:m0 + DH],
                            rhs=xm_t[:, r0 * W:(r0 + nr) * W],
                            start=True, stop=True)
                        dst = bass.AP(
                            tensor=hp.tensor,
                            offset=hp.offset + r0 * EXTW + 3,
                            ap=[list(hp.ap[0]), [EXTW, nr], [1, W]])
                        nc.scalar.copy(out=dst, in_=ps[:, 0:nr * W])
                    # nn=0 via PE diag-matmuls, nn=1 via a DVE FMA chain —
                    # the two halves run concurrently on different engines.
                    for nn, (r0, nr) in enumerate([(0, 8), (8, 8)]):
                        gsl = g_t[tt][:, nn * CH:nn * CH + nr * W]
                        if nn == 0:
                            pscv = pp.tile([DH, CH], F32, tag="bc2", bufs=2,
                                           name="cvps")
                            for k in range(9):
                                dy, dx = divmod(k, 3)
                                dy -= 1
                                dx -= 1
                                srcap = bass.AP(
                                    tensor=hp.tensor,
                                    offset=(hp.offset
                                            + (r0 + 3 + dy * dil) * EXTW
                                            + 3 + dx * dil),
                                    ap=[list(hp.ap[0]), [EXTW, nr], [1, W]])
                                nc.tensor.matmul(
                                    out=pscv[:, 0:nr * W],
                                    lhsT=dwj[:, (k * 2 + tt) * DH:
                                             (k * 2 + tt + 1) * DH],
                                    rhs=srcap, start=(k == 0), stop=(k == 8))
                            if j == 0:
                                nc.scalar.activation(out=gsl,
                                                     in_=pscv[:, 0:nr * W],
                                                     func=AF.Gelu)
                            else:
                                nc.vector.tensor_tensor(out=gsl,
                                                        in0=gsl.bitcast(F32),
                                                        in1=pscv[:, 0:nr * W],
                                                        op=OP.mult)
                            continue
                        acc = pt.tile([DH, CH], BF16, tag="cvav",
                                      name=f"cva{j}{tt}")
                        for k in range(9):
                            dy, dx = divmod(k, 3)
                            dy -= 1
                            dx -= 1
                            srcap = bass.AP(
                                tensor=hp.tensor,
                                offset=(hp.offset
                                        + (r0 + 3 + dy * dil) * EXTW
                                        + 3 + dx * dil),
                                ap=[list(hp.ap[0]), [EXTW, nr], [1, W]])
                            wcol = dwcol_t[:, (j * 2 + tt) * 9 + k:
                                           (j * 2 + tt) * 9 + k + 1]
                            if k == 0:
                                nc.vector.tensor_scalar_mul(acc[:, 0:nr * W],
                                                            srcap, wcol)
                            else:
                                nc.vector.scalar_tensor_tensor(
                                    out=acc[:, 0:nr * W], in0=srcap,
                                    scalar=wcol, in1=acc[:, 0:nr * W],
                                    op0=OP.mult, op1=OP.add)
                        if j == 0:
                            nc.scalar.activation(out=gsl, in_=acc[:, 0:nr * W],
                                                 func=AF.Gelu)
                        else:
                            nc.vector.tensor_tensor(out=gsl,
                                                    in0=gsl.bitcast(F32),
                                                    in1=acc[:, 0:nr * W],
                                                    op=OP.mult)

            if DEBUG:
                nc.sync.dma_start(out=dbg["g0"][:, :], in_=g_t[0].bitcast(F32))
                nc.sync.dma_start(out=dbg["g1"][:, :], in_=g_t[1].bitcast(F32))
            outt = slot("outt", [C, LQ], F32, "sl1")
            for nn in range(2):
                ps = pp.tile([C, CH], F32, tag="bc1", bufs=2, name="pops")
                nc.tensor.matmul(out=ps, lhsT=mwout_t[:, 0:C],
                                 rhs=g_t[0][:, nn * CH:(nn + 1) * CH],
                                 start=True, stop=False)
                nc.tensor.matmul(out=ps, lhsT=mwout_t[:, C:2 * C],
                                 rhs=g_t[1][:, nn * CH:(nn + 1) * CH],
                                 start=False, stop=True)
                nc.vector.scalar_tensor_tensor(
                    out=outt[:, nn * CH:(nn + 1) * CH], in0=ps,
                    scalar=g2_t[:, 0:1],
                    in1=xsl_t[:, 3 * W + nn * CH:3 * W + (nn + 1) * CH].bitcast(F32),
                    op0=OP.mult, op1=OP.add)
            nc.sync.dma_start(out=out_d[:, :], in_=outt[:, :])

    nc.compile()
    return nc


def _host_inputs(inputs):
    """Build the 8 per-core input maps."""
    x = inputs["x"].astype(np.float32)
    B = x.shape[0]
    maps = []
    ones96 = np.ones((1, C), np.float32)
    import ml_dtypes

    # packed-scan selectors: partition p = 16*dl + s; chunk j covers
    # d = 8j + dl.
    sel16 = np.zeros((80, 128), ml_dtypes.bfloat16)
    for p in range(128):
        sel16[32 + p % 16, p] = 1.0
        sel16[64 + p % 16, p] = 1.0
    sel96 = np.zeros((C, 12 * 128), np.float32)
    for j in range(12):
        for p in range(128):
            sel96[8 * j + p // 16, j * 128 + p] = 1.0
    selred = np.zeros((128, 12 * C), ml_dtypes.bfloat16)
    for j in range(12):
        for p in range(128):
            selred[p, j * C + 8 * j + p // 16] = 1.0
    statw = np.zeros((C, 4), np.float32)
    statw[:, 0] = 1.0   # K-tile0 col m=0 (sum x)
    statw[:, 3] = 1.0   # K-tile1 col m=1 (sum x^2)
    lnw = np.stack([inputs["ln1_b"], inputs["ln1_w"]]).astype(np.float32)
    w1 = inputs["ln1_w"].astype(np.float32)
    b1 = inputs["ln1_b"].astype(np.float32)
    w2 = inputs["mn_w"].astype(np.float32)
    b2 = inputs["mn_b"].astype(np.float32)
    stat6 = np.stack([np.ones(C, np.float32), w1, w1 * w1, w1 * b1,
                      np.ones(C, np.float32), w1 * w1], axis=1).copy()
    lnsc = np.zeros((128, 6), np.float32)
    lnsc[:, 0] = -w1.sum()
    lnsc[:, 1] = b1.sum() / C
    lnsc[:, 2] = (w1 * w1).sum()
    lnsc[:, 3] = -(w1 * b1).sum()
    lnsc[:, 4] = (b1 * b1).sum() / C
    w12row = (w1 * w2)[None, :].copy()
    wb4 = np.zeros((36, C), np.float32)
    wb4[32:36] = np.stack([-w1 * w2, w2 * b1, -w2, b2])
    lnwp = np.zeros((34, C), np.float32)
    lnwp[32:34] = np.stack([b1, w1])
    mwin = inputs["msff_win"].T.copy()          # [96, 576]
    mwout_full = inputs["msff_wout"].T          # [192, 96]
    mwout = np.concatenate([mwout_full[0:DH], mwout_full[DH:2 * DH]], axis=1).copy()
    dwcol = np.zeros((DH, 3, 2, 9), np.float32)
    dwdiag = np.zeros((DH, 3, 9, 2, DH), np.float32)
    for j, nm in enumerate(("msff_dw1", "msff_dw2", "msff_dw3")):
        wdw = inputs[nm]                        # [192, 3, 3]
        for k in range(9):
            dy, dx = divmod(k, 3)
            for tt in range(2):
                dwcol[:, j, tt, k] = wdw[tt * DH:(tt + 1) * DH, dy, dx]
                d = np.arange(DH)
                dwdiag[d, j, k, tt, d] = wdw[tt * DH + d, dy, dx]
    dwcol = dwcol.reshape(DH, 54).copy()
    dwdiag = dwdiag.reshape(DH, 3 * 9 * 2 * DH).astype(ml_dtypes.bfloat16)

    for c in range(8):
        b = c // 4
        dr = (c // 2) % 2
        hf = c % 2
        q = c % 4
        pfx = "f_" if dr == 0 else "b_"
        Win = inputs[pfx + "Win"].astype(np.float32)     # [384, 96]
        convw = inputs[pfx + "convw"].astype(np.float32)  # [192, 4]
        convb = inputs[pfx + "convb"].astype(np.float32)
        Wx = inputs[pfx + "Wx"].astype(np.float32)        # [38, 192]
        Wdt = inputs[pfx + "Wdt"].astype(np.float32)      # [192, 6]
        bdt = inputs[pfx + "bdt"].astype(np.float32)
        Alog = inputs[pfx + "Alog"].astype(np.float32)    # [192, 16]
        Dp = inputs[pfx + "D"].astype(np.float32)
        dperm = np.r_[hf * DH:(hf + 1) * DH, (1 - hf) * DH:(2 - hf) * DH]

        xfwd = np.ascontiguousarray(x[b].reshape(C, L))
        xin = xfwd[:, ::-1].copy() if dr == 1 else xfwd

        wconv = np.empty((C, 4, DI), np.float32)
        Win_c = Win[:DI]                                  # xc part [192, 96]
        for t in range(4):
            wconv[:, t, :] = (Win_c[dperm, :] * convw[dperm, t][:, None]).T
        wconv = wconv.reshape(C, 4 * DI).copy()
        winz = Win[DI + hf * DH:DI + (hf + 1) * DH, :].T.copy()
        convb2 = np.stack([convb[dperm[:DH]], convb[dperm[DH:]]], axis=1)
        wx_full = Wx[:, dperm].T                          # [192, 38]
        wx = np.zeros((DH, 160), np.float32)
        for j in range(2):
            blk = wx_full[j * DH:(j + 1) * DH]
            wx[:, j * 80 + 0:j * 80 + 6] = blk[:, 0:DTR]
            wx[:, j * 80 + 32:j * 80 + 48] = blk[:, DTR:DTR + DS]
            wx[:, j * 80 + 64:j * 80 + 80] = blk[:, DTR + DS:DTR + 2 * DS]
        wdt = Wdt[dperm[:DH], :].T.copy()                 # [6, 96]
        bdt_o = bdt[dperm[:DH]][:, None].copy()
        amat = -np.exp(Alog[dperm[:DH]])                  # [96, 16]
        apack = np.zeros((128, 12), np.float32)
        for j in range(12):
            for p in range(128):
                apack[p, j] = amat[8 * j + p // 16, p % 16]
        dpar = Dp[dperm[:DH]][:, None].copy()
        wout = inputs["Wout"].astype(np.float32)[:, dperm[:DH]].T.copy()

        m = np.full((C, 1), 1.0 if dr == 0 else 0.0, np.float32)

        dmask = np.zeros(ROWS_SL, np.float32)
        for rw in range(ROWS_SL):
            gr = 16 * q - 3 + rw
            if 0 <= gr < H:
                dmask[rw] = 1.0
        dmask2 = np.repeat(dmask, W)[None, :].repeat(2, 0).copy()
        dmask128 = np.repeat(dmask, W).reshape(128, LSL // 128).copy()
        qoff = np.array([[q * 16 * W]], np.int32)

        maps.append({
            "xin": np.ascontiguousarray(xin),
            "xfwd": xfwd,
            "wconv": wconv, "winz": winz, "convb": convb2,
            "wx": wx, "wdt": wdt, "bdt": bdt_o, "apack": apack,
            "dpar": dpar, "wout": wout,
            "lnw": lnw, "lnwc": lnw[1:2].copy(),
            "statw": statw, "stat6": stat6, "lnsc": lnsc,
            "w12row": w12row, "wb4": wb4, "lnwp": lnwp, "ones96": ones96,
            "sel16": sel16, "sel96": sel96, "selred": selred,
            "mask": m, "maskb": (1.0 - m).copy(),
            "gam1": inputs["gamma1"].astype(np.float32)[:, None].copy(),
            "gam2": inputs["gamma2"].astype(np.float32)[:, None].copy(),
            "mwin": mwin, "mwout": mwout, "dwcol": dwcol,
            "dwdiag": dwdiag,
            "dmask2": np.ascontiguousarray(dmask2),
            "onesrow": np.ones((1, L), np.float32),
            "dmask128": dmask128,
            "qoff": qoff,
        })
    return maps


def kernel(**inputs) -> np.ndarray:
    from concourse.bass_utils import run_bass_kernel_spmd

    if "nc" not in _COMPILED:
        _COMPILED["nc"] = _build()
    nc = _COMPILED["nc"]
    maps = _host_inputs(inputs)
    res = run_bass_kernel_spmd(nc, maps, core_ids=list(range(8)))
    out = np.empty((2, C, H, W), np.float32)
    for c in range(8):
        b = c // 4
        q = c % 4
        out[b, :, 16 * q:16 * (q + 1), :] = \
            res.results[c]["out"].reshape(C, 16, W)
    return out



# revision 63
# speedup vs baseline: 13.0873x; 13.0873x over previous
"""Trainium2 Bass kernel for nn_Block_5617817223712 (BiPixelMamba + MSFF block).

Sharding: 8 cores = (batch 2) x (scan direction 2) x (d_inner half 2).
Each core runs the full LN1/LN2 prologue on its (possibly reversed) batch
image, the Mamba branch for its direction and d_inner half (selective scan
via the DVE tensor_tensor_scan instruction, per-state-index tiles with the
exp(delta*A) computed on ACT with per-partition scale), a 4-way AllReduce of
the out_proj partials, and then the MSFF branch on its quarter of the rows.

The SPMD program is identical on all cores; per-core behavior (which batch,
direction reversal, d-half weight slices, row-quarter) is driven entirely by
per-core input data (sliced/reversed/permuted on the host).
"""
import sys

sys.path.insert(0, "/opt/trn_rl_repo")

import numpy as np

C = 96          # d_model channels
H = 64
W = 64
L = H * W       # 4096 pixels
DI = 192        # d_inner
DH = 96         # d_inner half per core
DS = 16         # d_state
DTR = 6         # dt_rank
HID = 192       # msff hidden
CH = 512        # free-dim chunk (one PSUM bank of fp32)
NCH = L // CH   # 8
ROWS_SL = 22    # stage-D slice rows (16 + 2*3 halo)
LSL = ROWS_SL * W   # 1408
LQ = 16 * W         # 1024 valid out pixels per core
EXTW = W + 6        # 70, W-padded row width for dwconv input
EPS1 = 1e-6
EPS2 = 1e-5

_COMPILED = {}
DEBUG = False


def _chunks(total, size):
    out = []
    off = 0
    while off < total:
        out.append((off, min(size, total - off)))
        off += size
    return out


def _build():
    import concourse.bass as bass
    import concourse.bacc as bacc
    import concourse.mybir as mybir
    from concourse.tile import TileContext

    F32 = mybir.dt.float32
    F32R = mybir.dt.float32r
    BF16 = mybir.dt.bfloat16
    I32 = mybir.dt.int32
    AF = mybir.ActivationFunctionType
    OP = mybir.AluOpType

    nc = bacc.Bacc("TRN2", target_bir_lowering=False, debug=False, num_devices=8)

    def din(name, shape, dt=F32):
        return nc.declare_dram_parameter(name, list(shape), dt, isOutput=False)

    # ---- inputs (per-core data prepared on host) ----
    xin_d = din("xin", [C, L])
    xfwd_d = din("xfwd", [C, L])
    wconv_d = din("wconv", [C, 4 * DI])
    winz_d = din("winz", [C, DH])
    convb_d = din("convb", [DH, 2])
    wx_d = din("wx", [DH, 160])
    sel16_d = din("sel16", [80, 128], mybir.dt.bfloat16)
    sel96_d = din("sel96", [C, 12 * 128])
    selred_d = din("selred", [128, 12 * C], mybir.dt.bfloat16)
    apack_d = din("apack", [128, 12])
    wdt_d = din("wdt", [DTR, DH])
    bdt_d = din("bdt", [DH, 1])
    dpar_d = din("dpar", [DH, 1])
    wout_d = din("wout", [DH, C])
    lnw_d = din("lnw", [2, C])
    lnwc_d = din("lnwc", [1, C])
    stat_d = din("statw", [C, 4])
    stat6_d = din("stat6", [C, 6])
    lnsc_d = din("lnsc", [128, 6])
    w12row_d = din("w12row", [1, C])
    wb4_d = din("wb4", [36, C])
    lnwp_d = din("lnwp", [34, C])
    dwdiag_d = din("dwdiag", [DH, 3 * 9 * 2 * DH], mybir.dt.bfloat16)
    mask_d = din("mask", [C, 1])
    maskb_d = din("maskb", [C, 1])
    g1_d = din("gam1", [C, 1])
    g2_d = din("gam2", [C, 1])
    mwin_d = din("mwin", [C, 3 * HID])
    mwout_d = din("mwout", [DH, 2 * C])
    dwcol_d = din("dwcol", [DH, 54])
    dmask2_d = din("dmask2", [2, LSL])
    dmask128_d = din("dmask128", [128, LSL // 128])
    qoff_d = din("qoff", [1, 1], I32)
    onesrow_d = din("onesrow", [1, L])

    out_d = nc.declare_dram_parameter("out", [C, LQ], F32, isOutput=True)
    dbg = {}
    if DEBUG:
        for nm, shape in (("x1", [C, L]), ("xn", [C, L]), ("xs0", [C, L]),
                          ("zs", [C, L]), ("dbl", [80, L]), ("delta", [C, L]),
                          ("y", [C, L]), ("yb", [C, L]), ("yo", [C, L]),
                          ("xme", [C, (H + 6) * W]), ("xsl", [C, LSL]),
                          ("xm", [C, LSL]), ("g0", [DH, LQ]), ("g1", [DH, LQ]),
                          ("hh0", [C, L]), ("dA0", [C, L]), ("dBx0", [C, L])):
            dbg[nm] = nc.declare_dram_parameter("dbg_" + nm, shape, F32,
                                                isOutput=True)

    yo_in_b = nc.dram_tensor("yo_in_b", [C, L], BF16)
    yo_out_b = nc.dram_tensor("yo_out_b", [C, L], BF16)

    SL_BYTES = (H + 6) * W  # largest slot free-elems (xme) = 4480

    with TileContext(nc) as tc:
        with tc.tile_pool(name="pw", bufs=1) as pw, \
             tc.tile_pool(name="pm", bufs=1) as pm, \
             tc.tile_pool(name="pt", bufs=2) as pt, \
             tc.tile_pool(name="pp", bufs=1, space="PSUM") as pp:

            def slot(name, shape, dt, tag):
                return pm.tile(list(shape), dt, name=name, tag=tag)

            # ---------- weights ----------
            def wtile(dram, shape, dt=F32):
                t = pw.tile(list(shape), dt, name="w_" + dram.name)
                s = dram[tuple(slice(None) for _ in shape)]
                if dt == F32R:
                    s = s.bitcast(F32R)
                nc.sync.dma_start(out=t, in_=s)
                return t

            # xin first so the LN stats pipeline starts before the ~4MB of
            # weight DMAs finish.
            xin_t = slot("xin", [C, L], F32R, "sl0")
            nc.sync.dma_start(out=xin_t, in_=xin_d[:, :].bitcast(F32R))

            wconv_t = wtile(wconv_d, [C, 4 * DI], F32R)
            winz_t = wtile(winz_d, [C, DH], F32R)
            convb_t = wtile(convb_d, [DH, 2])
            wx_t = wtile(wx_d, [DH, 160], F32R)
            sel16_t = wtile(sel16_d, [80, 128], BF16)
            sel96_t = wtile(sel96_d, [C, 12 * 128], F32R)
            selred_t = wtile(selred_d, [128, 12 * C], BF16)
            apack_t = wtile(apack_d, [128, 12])
            wdt_t = wtile(wdt_d, [DTR, DH], F32R)
            bdt_t = wtile(bdt_d, [DH, 1])
            dpar_t = wtile(dpar_d, [DH, 1])
            wout_t = wtile(wout_d, [DH, C], F32R)
            lnw_t = wtile(lnw_d, [2, C], F32R)
            lnwc_t = wtile(lnwc_d, [1, C], F32R)
            stat_t = wtile(stat_d, [C, 4], F32R)
            stat6_t = wtile(stat6_d, [C, 6], F32R)
            lnsc_t = wtile(lnsc_d, [128, 6])
            w12row_t = wtile(w12row_d, [1, C], F32R)
            wb4_t = wtile(wb4_d, [36, C], F32R)
            lnwp_t = wtile(lnwp_d, [34, C], F32R)
            mask_t = wtile(mask_d, [C, 1])
            maskb_t = wtile(maskb_d, [C, 1])
            g1_t = wtile(g1_d, [C, 1])
            g2_t = wtile(g2_d, [C, 1])
            mwin_t = wtile(mwin_d, [C, 3 * HID], F32R)
            mwout_t = wtile(mwout_d, [DH, 2 * C], F32R)
            dwcol_t = wtile(dwcol_d, [DH, 54])
            dmask2_t = wtile(dmask2_d, [2, LSL])
            dmask128_t = wtile(dmask128_d, [128, LSL // 128])
            qoff_t = wtile(qoff_d, [1, 1], I32)

            eps_t = pw.tile([128, 2], F32, name="eps_t")
            nc.vector.memset(eps_t[:, 0:1], EPS1)
            nc.vector.memset(eps_t[:, 1:2], EPS2)

            def rev(ap2d, Lx):
                return bass.AP(
                    tensor=ap2d.tensor,
                    offset=ap2d.offset + (Lx - 1),
                    ap=[list(ap2d.ap[0]), [-1, Lx]],
                )

            # ---------- LN helpers ----------
            def ln_stats(src_t, Lx, eps_col, rows_name):
                sq = slot("sq_" + rows_name, [C, Lx], F32R, "sl1")
                nc.scalar.activation(out=sq, in_=src_t[:, 0:Lx].bitcast(F32),
                                     func=AF.Square)
                S = slot("S_" + rows_name, [2, Lx], F32, "sl2")
                for off, sz in _chunks(Lx, CH):
                    ps = pp.tile([2, CH], F32, tag="bc1", bufs=2, name="lnps")
                    nc.tensor.matmul(out=ps[:, 0:sz], lhsT=stat_t[:, 0:2],
                                     rhs=src_t[:, off:off + sz],
                                     start=True, stop=False)
                    nc.tensor.matmul(out=ps[:, 0:sz], lhsT=stat_t[:, 2:4],
                                     rhs=sq[:, off:off + sz],
                                     start=False, stop=True)
                    nc.scalar.copy(out=S[:, off:off + sz], in_=ps[:, 0:sz])
                k = Lx // 128
                Sr = pt.tile([128, 2 * k], F32, tag="Sr", name="Sr")
                nc.sync.dma_start(
                    out=Sr[:, 0:k],
                    in_=S[0:1, :].rearrange("o (p k) -> o p k", k=k))
                nc.sync.dma_start(
                    out=Sr[:, k:2 * k],
                    in_=S[1:2, :].rearrange("o (p k) -> o p k", k=k))
                u = pt.tile([128, k], F32, tag="u_r", name="u_r")
                nc.vector.tensor_scalar_mul(u, Sr[:, 0:k], 1.0 / C)
                usq = pt.tile([128, k], F32, tag="usq_r", name="usq_r")
                nc.vector.tensor_tensor(out=usq, in0=u, in1=u, op=OP.mult)
                var = pt.tile([128, k], F32, tag="var_r", name="var_r")
                nc.vector.scalar_tensor_tensor(
                    out=var, in0=Sr[:, k:2 * k], scalar=1.0 / C, in1=usq,
                    op0=OP.mult, op1=OP.subtract)
                lnv = pt.tile([128, k], F32, tag="lnv_r", name="lnv_r")
                nc.scalar.activation(out=lnv, in_=var, func=AF.Ln,
                                     bias=eps_t[:, eps_col:eps_col + 1])
                rr = pt.tile([128, k], F32, tag="rr_r", name="rr_r")
                nc.scalar.activation(out=rr, in_=lnv, func=AF.Exp, scale=-0.5)
                ar = pt.tile([128, k], F32, tag="ar_r", name="ar_r")
                nc.vector.scalar_tensor_tensor(
                    out=ar, in0=u, scalar=-1.0, in1=rr, op0=OP.mult, op1=OP.mult)
                rows = slot(rows_name, [2, Lx], F32R, "sl3")
                nc.sync.dma_start(
                    out=rows[0:1, :].rearrange("o (p k) -> o p k", k=k),
                    in_=rr.bitcast(F32R))
                nc.sync.dma_start(
                    out=rows[1:2, :].rearrange("o (p k) -> o p k", k=k),
                    in_=ar.bitcast(F32R))
                return rows, rr, ar

            def ln_apply(src_t, r_src, ra_t, wc_t, wb_t, out_t, out_off, Lx):
                for off, sz in _chunks(Lx, CH):
                    rw = pp.tile([C, CH], F32, tag="bc1", bufs=2, name="rw")
                    nc.tensor.matmul(out=rw[:, 0:sz], lhsT=wc_t,
                                     rhs=r_src[0:1, off:off + sz],
                                     start=True, stop=True)
                    aw = pp.tile([C, CH], F32, tag="bc2", bufs=2, name="aw")
                    nc.tensor.matmul(out=aw[:, 0:sz], lhsT=wb_t,
                                     rhs=ra_t[:, off:off + sz],
                                     start=True, stop=True)
                    t = pt.tile([C, CH], F32, tag="lnt", name="lnt")
                    nc.vector.tensor_tensor(out=t[:, 0:sz],
                                            in0=src_t[:, off:off + sz].bitcast(F32),
                                            in1=rw[:, 0:sz], op=OP.mult)
                    nc.vector.tensor_tensor(
                        out=out_t[:, out_off + off:out_off + off + sz],
                        in0=t[:, 0:sz], in1=aw[:, 0:sz], op=OP.add)

            # ---------- stage A: fused double-LN ----------
            # Both LN1 (channels-first) and the inner LN (also over C) are
            # derived from a single stats pass over x: six weighted channel
            # sums {x, w1 x, w1^2 x, w1 b1 x} and {x^2, w1^2 x^2} give the
            # row stats of x1 = LN1(x) algebraically.
            sqa = slot("sqa", [C, L], F32R, "sl1")
            nc.scalar.activation(out=sqa, in_=xin_t.bitcast(F32),
                                 func=AF.Square)
            S6 = slot("S6", [34, L], F32, "sl2")
            for off, sz in _chunks(L, CH):
                ps1 = pp.tile([4, CH], F32, tag="bc1", bufs=2, name="st1")
                nc.tensor.matmul(out=ps1[:, 0:sz], lhsT=stat6_t[:, 0:4],
                                 rhs=xin_t[:, off:off + sz],
                                 start=True, stop=True)
                nc.scalar.copy(out=S6[0:4, off:off + sz], in_=ps1[:, 0:sz])
                ps2 = pp.tile([2, CH], F32, tag="bc2", bufs=2, name="st2")
                nc.tensor.matmul(out=ps2[:, 0:sz], lhsT=stat6_t[:, 4:6],
                                 rhs=sqa[:, off:off + sz],
                                 start=True, stop=True)
                nc.scalar.copy(out=S6[32:34, off:off + sz], in_=ps2[:, 0:sz])
            ka = L // 128
            R6 = pt.tile([128, 6 * ka], F32, tag="R6", name="R6", bufs=1)
            for i, sr in enumerate((0, 1, 2, 3, 32, 33)):
                nc.sync.dma_start(
                    out=R6[:, i * ka:(i + 1) * ka],
                    in_=S6[sr:sr + 1, :].rearrange("o (p k) -> o p k", k=ka))

            def row(name):
                return pt.tile([128, ka], F32, tag="row_" + name, name=name,
                               bufs=1)

            TT = nc.vector.tensor_tensor
            STT = nc.vector.scalar_tensor_tensor
            mu1 = row("mu1")
            nc.vector.tensor_scalar_mul(mu1, R6[:, 0:ka], 1.0 / C)
            usq1 = row("usq1")
            TT(out=usq1, in0=mu1, in1=mu1, op=OP.mult)
            var1 = row("var1")
            STT(out=var1, in0=R6[:, 4 * ka:5 * ka], scalar=1.0 / C, in1=usq1,
                op0=OP.mult, op1=OP.subtract)
            lnv1 = row("lnv1")
            nc.scalar.activation(out=lnv1, in_=var1, func=AF.Ln,
                                 bias=eps_t[:, 0:1])
            r1 = row("r1")
            nc.scalar.activation(out=r1, in_=lnv1, func=AF.Exp, scale=-0.5)
            r1mu1 = row("r1mu1")
            TT(out=r1mu1, in0=r1, in1=mu1, op=OP.mult)
            rA = row("rA")
            TT(out=rA, in0=r1, in1=R6[:, ka:2 * ka], op=OP.mult)
            S2r = row("S2r")
            STT(out=S2r, in0=r1mu1, scalar=lnsc_t[:, 0:1], in1=rA,
                op0=OP.mult, op1=OP.add)
            mu2 = row("mu2")
            nc.scalar.activation(out=mu2, in_=S2r, func=AF.Identity,
                                 scale=1.0 / C, bias=lnsc_t[:, 1:2])
            m2D = row("m2D")
            TT(out=m2D, in0=mu1, in1=R6[:, 2 * ka:3 * ka], op=OP.mult)
            T1a = row("T1a")
            STT(out=T1a, in0=m2D, scalar=-2.0, in1=R6[:, 5 * ka:6 * ka],
                op0=OP.mult, op1=OP.add)
            T1 = row("T1")
            STT(out=T1, in0=usq1, scalar=lnsc_t[:, 2:3], in1=T1a,
                op0=OP.mult, op1=OP.add)
            T2 = row("T2")
            STT(out=T2, in0=mu1, scalar=lnsc_t[:, 3:4], in1=R6[:, 3 * ka:4 * ka],
                op0=OP.mult, op1=OP.add)
            r1sq = row("r1sq")
            TT(out=r1sq, in0=r1, in1=r1, op=OP.mult)
            Q2a = row("Q2a")
            TT(out=Q2a, in0=r1sq, in1=T1, op=OP.mult)
            Q2b = row("Q2b")
            TT(out=Q2b, in0=r1, in1=T2, op=OP.mult)
            Q2 = row("Q2")
            STT(out=Q2, in0=Q2b, scalar=2.0, in1=Q2a, op0=OP.mult, op1=OP.add)
            q2c = row("q2c")
            nc.scalar.activation(out=q2c, in_=Q2, func=AF.Identity,
                                 scale=1.0 / C, bias=lnsc_t[:, 4:5])
            usq2 = row("usq2")
            TT(out=usq2, in0=mu2, in1=mu2, op=OP.mult)
            var2 = row("var2")
            TT(out=var2, in0=q2c, in1=usq2, op=OP.subtract)
            lnv2 = row("lnv2")
            nc.scalar.activation(out=lnv2, in_=var2, func=AF.Ln,
                                 bias=eps_t[:, 1:2])
            r2 = row("r2")
            nc.scalar.activation(out=r2, in_=lnv2, func=AF.Exp, scale=-0.5)
            Pr = row("Pr")
            TT(out=Pr, in0=r1, in1=r2, op=OP.mult)
            Pmu1 = row("Pmu1")
            TT(out=Pmu1, in0=Pr, in1=mu1, op=OP.mult)
            r2mu2 = row("r2mu2")
            TT(out=r2mu2, in0=r2, in1=mu2, op=OP.mult)
            ar1 = row("ar1")
            STT(out=ar1, in0=mu1, scalar=-1.0, in1=r1, op0=OP.mult, op1=OP.mult)

            rxn = slot("rxn", [36, L], F32R, "sl3")
            for i, rt in ((0, Pr), (32, Pmu1), (33, r2), (34, r2mu2)):
                nc.sync.dma_start(
                    out=rxn[i:i + 1, :].rearrange("o (p k) -> o p k", k=ka),
                    in_=rt.bitcast(F32R))
            nc.sync.dma_start(out=rxn[35:36, :],
                              in_=onesrow_d[:, 0:L].bitcast(F32R))
            rx1 = slot("rx1", [34, L], F32R, "sl4")
            for i, rt in ((0, r1), (33, ar1)):
                nc.sync.dma_start(
                    out=rx1[i:i + 1, :].rearrange("o (p k) -> o p k", k=ka),
                    in_=rt.bitcast(F32R))
            nc.sync.dma_start(out=rx1[32:33, :],
                              in_=onesrow_d[:, 0:L].bitcast(F32R))

            xn_t = slot("xn", [C, L + 3], F32R, "sl6")
            nc.vector.memset(xn_t[:, 0:3].bitcast(F32), 0.0)
            ln_apply(xin_t, rxn[0:1, :], rxn[32:36, :], w12row_t,
                     wb4_t[32:36, :], xn_t, 3, L)
            x1_t = slot("x1", [C, L], F32R, "sl5")
            ln_apply(xin_t, rx1[0:1, :], rx1[32:34, :], lnwc_t,
                     lnwp_t[32:34, :], x1_t, 0, L)

            # ---------- stage B ----------
            xs0_t = slot("xs0", [C, L], F32R, "sl0")
            xs1_t = slot("xs1", [C, L], F32R, "sl1")
            zs_t = slot("zs", [C, L], BF16, "sl2")
            for off, sz in _chunks(L, CH):
                for mt, xs_t in ((0, xs0_t), (1, xs1_t)):
                    ps = pp.tile([C, CH], F32, tag="bc1", bufs=2, name="cps")
                    for t in range(4):
                        nc.tensor.matmul(
                            out=ps[:, 0:sz],
                            lhsT=wconv_t[:, t * DI + mt * DH:t * DI + (mt + 1) * DH],
                            rhs=xn_t[:, off + t:off + t + sz],
                            start=(t == 0), stop=(t == 3))
                    nc.scalar.activation(out=xs_t[:, off:off + sz], in_=ps[:, 0:sz],
                                         func=AF.Silu, bias=convb_t[:, mt:mt + 1])
                psz = pp.tile([C, CH], F32, tag="bc2", bufs=2, name="zps")
                nc.tensor.matmul(out=psz[:, 0:sz], lhsT=winz_t,
                                 rhs=xn_t[:, 3 + off:3 + off + sz],
                                 start=True, stop=True)
                nc.scalar.activation(out=zs_t[:, off:off + sz], in_=psz[:, 0:sz],
                                     func=AF.Silu)

            dbl_t = slot("dbl", [80, L], F32R, "sl3")
            dblbf_t = pw.tile([80, L], BF16, name="dblbf")
            for off, sz in _chunks(L, CH):
                ps = pp.tile([80, CH], F32, tag="bc1", bufs=2, name="dps")
                nc.tensor.matmul(out=ps[:, 0:sz], lhsT=wx_t[:, 0:80],
                                 rhs=xs0_t[:, off:off + sz], start=True, stop=False)
                nc.tensor.matmul(out=ps[:, 0:sz], lhsT=wx_t[:, 80:160],
                                 rhs=xs1_t[:, off:off + sz], start=False, stop=True)
                nc.scalar.copy(out=dbl_t[:, off:off + sz], in_=ps[:, 0:sz])
                nc.vector.tensor_copy(out=dblbf_t[32:64, off:off + sz],
                                      in_=ps[32:64, 0:sz])
                nc.vector.tensor_copy(out=dblbf_t[64:80, off:off + sz],
                                      in_=ps[64:80, 0:sz])

            esp_t = slot("esp", [C, L], F32, "sl4")
            for off, sz in _chunks(L, CH):
                ps = pp.tile([C, CH], F32, tag="bc1", bufs=2, name="dtps")
                nc.tensor.matmul(out=ps[:, 0:sz], lhsT=wdt_t,
                                 rhs=dbl_t[0:DTR, off:off + sz],
                                 start=True, stop=True)
                nc.scalar.activation(out=esp_t[:, off:off + sz], in_=ps[:, 0:sz],
                                     func=AF.Exp, bias=bdt_t[:, 0:1])
            delta_t = slot("delta", [C, L], F32R, "sl6")
            nc.scalar.activation(out=delta_t, in_=esp_t, func=AF.Ln, bias=1.0)

            u_t = slot("u", [C, L], F32R, "sl1")
            nc.vector.tensor_tensor(out=u_t, in0=delta_t.bitcast(F32),
                                    in1=xs0_t.bitcast(F32), op=OP.mult)

            # ---------- packed selective scan ----------
            # (d, s) pairs packed onto 128 partitions: p = 16*dl + s, 12
            # chunks j covering d = 8j + dl. Broadcasts of delta/u into the
            # packed layout via 0/1-selector matmuls; dA = exp on Scalar with
            # per-partition A scale; scan on DVE; w = hh*C on gpsimd; the
            # sum over s via selector matmuls into resident PSUM banks.
            LH = L // 2
            NCHK = 12
            bbcc_t = slot("bbcc", [128, 2 * L], BF16, "sl3")
            for off, sz in _chunks(L, CH):
                pb = pp.tile([128, CH], F32, tag="bc1", bufs=2, name="pbb")
                nc.tensor.matmul(out=pb[:, 0:sz], lhsT=sel16_t[32:48, :],
                                 rhs=dblbf_t[32:48, off:off + sz],
                                 start=True, stop=True)
                nc.scalar.copy(out=bbcc_t[:, off:off + sz], in_=pb[:, 0:sz])
                pc = pp.tile([128, CH], F32, tag="bc2", bufs=2, name="pcc")
                nc.tensor.matmul(out=pc[:, 0:sz], lhsT=sel16_t[64:80, :],
                                 rhs=dblbf_t[64:80, off:off + sz],
                                 start=True, stop=True)
                nc.scalar.copy(out=bbcc_t[:, L + off:L + off + sz],
                               in_=pc[:, 0:sz])

            carry_t = pw.tile([128, NCHK], F32, name="carry_t")
            for hfl in range(2):
                ho = hfl * LH
                yh = slot(f"y{hfl}", [C, LH], F32R, "sl4")
                yps = [pp.tile([C, CH], F32, tag=f"yp{q}", bufs=1,
                               name=f"yps{hfl}{q}") for q in range(4)]
                for j in range(NCHK):
                    dA = slot(f"dA{hfl}_{j}", [128, LH], BF16,
                              "sl7" if j % 2 == 0 else "sl8")
                    dBx = slot(f"dBx{hfl}_{j}", [128, LH], BF16,
                               "sl9" if j % 2 == 0 else "slA")
                    for q in range(4):
                        co = ho + q * CH
                        psD = pp.tile([128, CH], F32, tag="bc1", bufs=2,
                                      name="psD")
                        nc.tensor.matmul(out=psD,
                                         lhsT=sel96_t[:, j * 128:(j + 1) * 128],
                                         rhs=delta_t[:, co:co + CH],
                                         start=True, stop=True)
                        nc.scalar.activation(out=dA[:, q * CH:(q + 1) * CH],
                                             in_=psD, func=AF.Exp,
                                             scale=apack_t[:, j:j + 1])
                        psU = pp.tile([128, CH], F32, tag="bc2", bufs=2,
                                      name="psU")
                        nc.tensor.matmul(out=psU,
                                         lhsT=sel96_t[:, j * 128:(j + 1) * 128],
                                         rhs=u_t[:, co:co + CH],
                                         start=True, stop=True)
                        nc.vector.tensor_tensor(out=dBx[:, q * CH:(q + 1) * CH],
                                                in0=psU,
                                                in1=bbcc_t[:, co:co + CH],
                                                op=OP.mult)
                    # reduce of the previous chunk goes to the PE queue after
                    # this chunk's broadcasts so chunk j+1 is never blocked
                    # behind w(j) on the in-order PE queue.
                    if j >= 1:
                        hhp = hh
                        for q in range(4):
                            nc.tensor.matmul(
                                out=yps[q],
                                lhsT=selred_t[:, (j - 1) * C:j * C],
                                rhs=hhp[:, q * CH:(q + 1) * CH],
                                start=(j == 1), stop=False)
                    hh = slot(f"hh{hfl}_{j}", [128, LH], BF16,
                              "slB" if j % 2 == 0 else "slC")
                    nc.vector.tensor_tensor_scan(
                        out=hh, data0=dA, data1=dBx,
                        initial=(0.0 if hfl == 0 else carry_t[:, j:j + 1]),
                        op0=OP.mult, op1=OP.add)
                    if hfl == 0:
                        nc.scalar.copy(out=carry_t[:, j:j + 1],
                                       in_=hh[:, LH - 1:LH])
                    nc.gpsimd.tensor_tensor(out=hh, in0=hh,
                                            in1=bbcc_t[:, L + ho:L + ho + LH],
                                            op=OP.mult)
                for q in range(4):
                    nc.tensor.matmul(out=yps[q],
                                     lhsT=selred_t[:, (NCHK - 1) * C:NCHK * C],
                                     rhs=hh[:, q * CH:(q + 1) * CH],
                                     start=False, stop=True)
                for q in range(4):
                    nc.vector.scalar_tensor_tensor(
                        out=yh[:, q * CH:(q + 1) * CH],
                        in0=xs0_t[:, ho + q * CH:ho + (q + 1) * CH].bitcast(F32),
                        scalar=dpar_t[:, 0:1], in1=yps[q],
                        op0=OP.mult, op1=OP.add)

                # gate + blend within this compute half. For dir=1 cores the
                # within-half reversal of compute-half hfl equals forward cols
                # of the OTHER half, so each core writes its content into the
                # appropriate bounce half (other half zeroed; AllReduce adds).
                nc.vector.tensor_tensor(out=yh, in0=yh.bitcast(F32),
                                        in1=zs_t[:, ho:ho + LH], op=OP.mult)
                t1h = slot(f"t1h{hfl}", [C, LH], BF16, "slT")
                rev_src = bass.AP(
                    tensor=yh.tensor,
                    offset=yh.offset + (LH - 1),
                    ap=[list(yh.ap[0]), [-1, LH]],
                )
                nc.vector.tensor_scalar_mul(t1h, rev_src.bitcast(F32),
                                            maskb_t[:, 0:1])
                ybh = slot(f"ybh{hfl}", [C, LH], F32R, "sl7")
                nc.vector.scalar_tensor_tensor(out=ybh, in0=yh.bitcast(F32),
                                               scalar=mask_t[:, 0:1],
                                               in1=t1h, op0=OP.mult, op1=OP.add)
                # masked dual writes: own-direction half gets data, other
                # zeros — masking folded into the PSUM->SBUF copies.
                zm = slot(f"zm{hfl}", [C, LH], BF16, "sl8")
                zmb = slot(f"zmb{hfl}", [C, LH], BF16, "slZ")
                for off, sz in _chunks(LH, CH):
                    ps = pp.tile([C, CH], F32, tag="bc1", bufs=2, name="ops")
                    nc.tensor.matmul(out=ps[:, 0:sz], lhsT=wout_t,
                                     rhs=ybh[:, off:off + sz],
                                     start=True, stop=True)
                    nc.scalar.activation(out=zm[:, off:off + sz],
                                         in_=ps[:, 0:sz], func=AF.Copy,
                                         scale=mask_t[:, 0:1])
                    nc.scalar.activation(out=zmb[:, off:off + sz],
                                         in_=ps[:, 0:sz], func=AF.Copy,
                                         scale=maskb_t[:, 0:1])
                nc.sync.dma_start(out=yo_in_b[:, ho:ho + LH], in_=zm[:, :])
                nc.sync.dma_start(out=yo_in_b[:, L - LH - ho:L - ho], in_=zmb[:, :])

            # ---------- stage C prep (independent of yo: overlaps the
            # AllReduce on the in-order engine queues) ----------
            t2 = slot("t2", [C, L], F32, "sl0")
            nc.vector.tensor_scalar_mul(t2, rev(x1_t[:, :].bitcast(F32), L),
                                        maskb_t[:, 0:1])
            x1c_t = slot("x1c", [C, L], F32, "sl1")
            nc.vector.scalar_tensor_tensor(out=x1c_t, in0=x1_t.bitcast(F32),
                                           scalar=mask_t[:, 0:1],
                                           in1=t2, op0=OP.mult, op1=OP.add)
            xfwd_t = slot("xfwd", [C, L], F32, "sl2")
            nc.sync.dma_start(out=xfwd_t, in_=xfwd_d[:, :])
            q_t = slot("q", [C, L], F32, "sl3")
            nc.vector.scalar_tensor_tensor(out=q_t, in0=x1c_t,
                                           scalar=g1_t[:, 0:1],
                                           in1=xfwd_t, op0=OP.mult, op1=OP.add)
            xme_t = slot("xme", [C, (H + 6) * W], F32, "sl6")
            nc.gpsimd.memset(xme_t[:, 0:3 * W], 0.0)
            nc.gpsimd.memset(xme_t[:, (H + 3) * W:(H + 6) * W], 0.0)

            nc.gpsimd.collective_compute(
                "AllReduce", OP.add,
                replica_groups=[[0, 1, 2, 3], [4, 5, 6, 7]],
                ins=[yo_in_b.ap().opt()],
                outs=[yo_out_b.ap().opt()],
            )
            yo_t = slot("yo", [C, L], BF16, "sl9")
            nc.sync.dma_start(out=yo_t, in_=yo_out_b[:, :])

            if DEBUG:
                nc.sync.dma_start(out=dbg["yo"][:, :], in_=yo_t[:, :])
            # ---------- stage C ----------
            nc.vector.scalar_tensor_tensor(
                out=xme_t[:, 3 * W:(H + 3) * W], in0=yo_t, scalar=g1_t[:, 0:1],
                in1=q_t, op0=OP.mult, op1=OP.add)

            # ---------- stage D ----------
            reg = nc.alloc_registers()
            nc.regs_load(reg, qoff_t[0:1, 0:1])
            qoff = nc.snap(reg, min_val=0, max_val=3 * 1024)
            xsl_t = slot("xsl", [C, LSL], F32R, "sl0")
            nc.scalar.copy(out=xsl_t, in_=xme_t[:, bass.ds(qoff, LSL)])

            if DEBUG:
                nc.sync.dma_start(out=dbg["xme"][:, :], in_=xme_t[:, :])
                nc.sync.dma_start(out=dbg["xsl"][:, :], in_=xsl_t.bitcast(F32))
            rows3, rr3, ar3 = ln_stats(xsl_t, LSL, 0, "rows3")
            k3 = LSL // 128
            rmr = pt.tile([128, k3], F32, tag="rmr", name="rmr")
            nc.vector.tensor_tensor(out=rmr, in0=rr3, in1=dmask128_t, op=OP.mult)
            amr = pt.tile([128, k3], F32, tag="amr", name="amr")
            nc.vector.tensor_tensor(out=amr, in0=ar3, in1=dmask128_t, op=OP.mult)
            rm = slot("rm", [1, LSL], F32R, "sl8")
            nc.sync.dma_start(
                out=rm[0:1, :].rearrange("o (p k) -> o p k", k=k3),
                in_=rmr.bitcast(F32R))
            ra3 = slot("ra3", [2, LSL], F32R, "sl4")
            nc.sync.dma_start(out=ra3[0:1, :],
                              in_=dmask2_t[0:1, :].bitcast(F32R))
            nc.sync.dma_start(
                out=ra3[1:2, :].rearrange("o (p k) -> o p k", k=k3),
                in_=amr.bitcast(F32R))
            xm_t = slot("xm", [C, LSL], F32R, "sl5")
            ln_apply(xsl_t, rm, ra3, lnwc_t, lnw_t, xm_t, 0, LSL)

            if DEBUG:
                nc.sync.dma_start(out=dbg["xm"][:, :], in_=xm_t.bitcast(F32))
            g_t = [slot(f"g{tt}", [DH, LQ], F32R, f"sl{2 + tt}")
                   for tt in range(2)]
            row_chunks = [(0, 8), (8, 8), (16, 6)]
            # depthwise 3x3 conv as 9 chained per-partition-scalar FMAs on
            # DVE/Pool (the weights are diagonal — no matmuls needed).
            for j in range(3):
                dil = j + 1
                dwj = pw.tile([DH, 9 * 2 * DH], BF16, tag="dwj", bufs=1,
                              name=f"dwj{j}")
                nc.sync.dma_start(
                    out=dwj,
                    in_=dwdiag_d[:, j * 9 * 2 * DH:(j + 1) * 9 * 2 * DH])
                for tt in range(2):
                    hp = slot(f"hp{j}{tt}", [DH, ROWS_SL * EXTW], BF16,
                              f"sl{7 + tt}")
                    nc.gpsimd.memset(hp[:, :], 0.0)
                    m0 = j * HID + tt * DH
                    for r0, nr in row_chunks:
                        ps = pp.tile([DH, CH], F32, tag="bc1", bufs=2, name="pips")
                        nc.tensor.matmul(
                            out=ps[:, 0:nr * W], lhsT=mwin_t[:, m0:m0 + DH],
                            rhs=xm_t[:, r0 * W:(r0 + nr) * W],
                            start=True, stop=True)
                        dst = bass.AP(
                            tensor=hp.tensor,
                            offset=hp.offset + r0 * EXTW + 3,
                            ap=[list(hp.ap[0]), [EXTW, nr], [1, W]])
                        nc.scalar.copy(out=dst, in_=ps[:, 0:nr * W])
                    # nn=0 via PE diag-matmuls, nn=1 via a DVE FMA chain —
                    # the two halves run concurrently on different engines.
                    for nn, (r0, nr) in enumerate([(0, 8), (8, 8)]):
                        gsl = g_t[tt][:, nn * CH:nn * CH + nr * W]
                        if nn == 0:
                            pscv = pp.tile([DH, CH], F32, tag="bc2", bufs=2,
                                           name="cvps")
                            for k in range(9):
                                dy, dx = divmod(k, 3)
                                dy -= 1
                                dx -= 1
                                srcap = bass.AP(
                                    tensor=hp.tensor,
                                    offset=(hp.offset
                                            + (r0 + 3 + dy * dil) * EXTW
                                            + 3 + dx * dil),
                                    ap=[list(hp.ap[0]), [EXTW, nr], [1, W]])
                                nc.tensor.matmul(
                                    out=pscv[:, 0:nr * W],
                                    lhsT=dwj[:, (k * 2 + tt) * DH:
                                             (k * 2 + tt + 1) * DH],
                                    rhs=srcap, start=(k == 0), stop=(k == 8))
                            if j == 0:
                                nc.scalar.activation(out=gsl,
                                                     in_=pscv[:, 0:nr * W],
                                                     func=AF.Gelu)
                            else:
                                nc.vector.tensor_tensor(out=gsl,
                                                        in0=gsl.bitcast(F32),
                                                        in1=pscv[:, 0:nr * W],
                                                        op=OP.mult)
                            continue
                        acc = pt.tile([DH, CH], BF16, tag="cvav",
                                      name=f"cva{j}{tt}")
                        for k in range(9):
                            dy, dx = divmod(k, 3)
                            dy -= 1
                            dx -= 1
                            srcap = bass.AP(
                                tensor=hp.tensor,
                                offset=(hp.offset
                                        + (r0 + 3 + dy * dil) * EXTW
                                        + 3 + dx * dil),
                                ap=[list(hp.ap[0]), [EXTW, nr], [1, W]])
                            wcol = dwcol_t[:, (j * 2 + tt) * 9 + k:
                                           (j * 2 + tt) * 9 + k + 1]
                            if k == 0:
                                nc.vector.tensor_scalar_mul(acc[:, 0:nr * W],
                                                            srcap, wcol)
                            else:
                                nc.vector.scalar_tensor_tensor(
                                    out=acc[:, 0:nr * W], in0=srcap,
                                    scalar=wcol, in1=acc[:, 0:nr * W],
                                    op0=OP.mult, op1=OP.add)
                        if j == 0:
                            nc.scalar.activation(out=gsl, in_=acc[:, 0:nr * W],
                                                 func=AF.Gelu)
                        else:
                            nc.vector.tensor_tensor(out=gsl,
                                                    in0=gsl.bitcast(F32),
                                                    in1=acc[:, 0:nr * W],
                                                    op=OP.mult)

            if DEBUG:
                nc.sync.dma_start(out=dbg["g0"][:, :], in_=g_t[0].bitcast(F32))
                nc.sync.dma_start(out=dbg["g1"][:, :], in_=g_t[1].bitcast(F32))
            outt = slot("outt", [C, LQ], F32, "sl1")
            for nn in range(2):
                ps = pp.tile([C, CH], F32, tag="bc1", bufs=2, name="pops")
                nc.tensor.matmul(out=ps, lhsT=mwout_t[:, 0:C],
                                 rhs=g_t[0][:, nn * CH:(nn + 1) * CH],
                                 start=True, stop=False)
                nc.tensor.matmul(out=ps, lhsT=mwout_t[:, C:2 * C],
                                 rhs=g_t[1][:, nn * CH:(nn + 1) * CH],
                                 start=False, stop=True)
                nc.vector.scalar_tensor_tensor(
                    out=outt[:, nn * CH:(nn + 1) * CH], in0=ps,
                    scalar=g2_t[:, 0:1],
                    in1=xsl_t[:, 3 * W + nn * CH:3 * W + (nn + 1) * CH].bitcast(F32),
                    op0=OP.mult, op1=OP.add)
            nc.sync.dma_start(out=out_d[:, :], in_=outt[:, :])

    nc.compile()
    return nc


def _host_inputs(inputs):
    """Build the 8 per-core input maps."""
    x = inputs["x"].astype(np.float32)
    B = x.shape[0]
    maps = []
    ones96 = np.ones((1, C), np.float32)
    import ml_dtypes

    # packed-scan selectors: partition p = 16*dl + s; chunk j covers
    # d = 8j + dl.
    sel16 = np.zeros((80, 128), ml_dtypes.bfloat16)
    for p in range(128):
        sel16[32 + p % 16, p] = 1.0
        sel16[64 + p % 16, p] = 1.0
    sel96 = np.zeros((C, 12 * 128), np.float32)
    for j in range(12):
        for p in range(128):
            sel96[8 * j + p // 16, j * 128 + p] = 1.0
    selred = np.zeros((128, 12 * C), ml_dtypes.bfloat16)
    for j in range(12):
        for p in range(128):
            selred[p, j * C + 8 * j + p // 16] = 1.0
    statw = np.zeros((C, 4), np.float32)
    statw[:, 0] = 1.0   # K-tile0 col m=0 (sum x)
    statw[:, 3] = 1.0   # K-tile1 col m=1 (sum x^2)
    lnw = np.stack([inputs["ln1_b"], inputs["ln1_w"]]).astype(np.float32)
    w1 = inputs["ln1_w"].astype(np.float32)
    b1 = inputs["ln1_b"].astype(np.float32)
    w2 = inputs["mn_w"].astype(np.float32)
    b2 = inputs["mn_b"].astype(np.float32)
    stat6 = np.stack([np.ones(C, np.float32), w1, w1 * w1, w1 * b1,
                      np.ones(C, np.float32), w1 * w1], axis=1).copy()
    lnsc = np.zeros((128, 6), np.float32)
    lnsc[:, 0] = -w1.sum()
    lnsc[:, 1] = b1.sum() / C
    lnsc[:, 2] = (w1 * w1).sum()
    lnsc[:, 3] = -(w1 * b1).sum()
    lnsc[:, 4] = (b1 * b1).sum() / C
    w12row = (w1 * w2)[None, :].copy()
    wb4 = np.zeros((36, C), np.float32)
    wb4[32:36] = np.stack([-w1 * w2, w2 * b1, -w2, b2])
    lnwp = np.zeros((34, C), np.float32)
    lnwp[32:34] = np.stack([b1, w1])
    mwin = inputs["msff_win"].T.copy()          # [96, 576]
    mwout_full = inputs["msff_wout"].T          # [192, 96]
    mwout = np.concatenate([mwout_full[0:DH], mwout_full[DH:2 * DH]], axis=1).copy()
    dwcol = np.zeros((DH, 3, 2, 9), np.float32)
    dwdiag = np.zeros((DH, 3, 9, 2, DH), np.float32)
    for j, nm in enumerate(("msff_dw1", "msff_dw2", "msff_dw3")):
        wdw = inputs[nm]                        # [192, 3, 3]
        for k in range(9):
            dy, dx = divmod(k, 3)
            for tt in range(2):
                dwcol[:, j, tt, k] = wdw[tt * DH:(tt + 1) * DH, dy, dx]
                d = np.arange(DH)
                dwdiag[d, j, k, tt, d] = wdw[tt * DH + d, dy, dx]
    dwcol = dwcol.reshape(DH, 54).copy()
    dwdiag = dwdiag.reshape(DH, 3 * 9 * 2 * DH).astype(ml_dtypes.bfloat16)

    for c in range(8):
        b = c // 4
        dr = (c // 2) % 2
        hf = c % 2
        q = c % 4
        pfx = "f_" if dr == 0 else "b_"
        Win = inputs[pfx + "Win"].astype(np.float32)     # [384, 96]
        convw = inputs[pfx + "convw"].astype(np.float32)  # [192, 4]
        convb = inputs[pfx + "convb"].astype(np.float32)
        Wx = inputs[pfx + "Wx"].astype(np.float32)        # [38, 192]
        Wdt = inputs[pfx + "Wdt"].astype(np.float32)      # [192, 6]
        bdt = inputs[pfx + "bdt"].astype(np.float32)
        Alog = inputs[pfx + "Alog"].astype(np.float32)    # [192, 16]
        Dp = inputs[pfx + "D"].astype(np.float32)
        dperm = np.r_[hf * DH:(hf + 1) * DH, (1 - hf) * DH:(2 - hf) * DH]

        xfwd = np.ascontiguousarray(x[b].reshape(C, L))
        xin = xfwd[:, ::-1].copy() if dr == 1 else xfwd

        wconv = np.empty((C, 4, DI), np.float32)
        Win_c = Win[:DI]                                  # xc part [192, 96]
        for t in range(4):
            wconv[:, t, :] = (Win_c[dperm, :] * convw[dperm, t][:, None]).T
        wconv = wconv.reshape(C, 4 * DI).copy()
        winz = Win[DI + hf * DH:DI + (hf + 1) * DH, :].T.copy()
        convb2 = np.stack([convb[dperm[:DH]], convb[dperm[DH:]]], axis=1)
        wx_full = Wx[:, dperm].T                          # [192, 38]
        wx = np.zeros((DH, 160), np.float32)
        for j in range(2):
            blk = wx_full[j * DH:(j + 1) * DH]
            wx[:, j * 80 + 0:j * 80 + 6] = blk[:, 0:DTR]
            wx[:, j * 80 + 32:j * 80 + 48] = blk[:, DTR:DTR + DS]
            wx[:, j * 80 + 64:j * 80 + 80] = blk[:, DTR + DS:DTR + 2 * DS]
        wdt = Wdt[dperm[:DH], :].T.copy()                 # [6, 96]
        bdt_o = bdt[dperm[:DH]][:, None].copy()
        amat = -np.exp(Alog[dperm[:DH]])                  # [96, 16]
        apack = np.zeros((128, 12), np.float32)
        for j in range(12):
            for p in range(128):
                apack[p, j] = amat[8 * j + p // 16, p % 16]
        dpar = Dp[dperm[:DH]][:, None].copy()
        wout = inputs["Wout"].astype(np.float32)[:, dperm[:DH]].T.copy()

        m = np.full((C, 1), 1.0 if dr == 0 else 0.0, np.float32)

        dmask = np.zeros(ROWS_SL, np.float32)
        for rw in range(ROWS_SL):
            gr = 16 * q - 3 + rw
            if 0 <= gr < H:
                dmask[rw] = 1.0
        dmask2 = np.repeat(dmask, W)[None, :].repeat(2, 0).copy()
        dmask128 = np.repeat(dmask, W).reshape(128, LSL // 128).copy()
        qoff = np.array([[q * 16 * W]], np.int32)

        maps.append({
            "xin": np.ascontiguousarray(xin),
            "xfwd": xfwd,
            "wconv": wconv, "winz": winz, "convb": convb2,
            "wx": wx, "wdt": wdt, "bdt": bdt_o, "apack": apack,
            "dpar": dpar, "wout": wout,
            "lnw": lnw, "lnwc": lnw[1:2].copy(),
            "statw": statw, "stat6": stat6, "lnsc": lnsc,
            "w12row": w12row, "wb4": wb4, "lnwp": lnwp, "ones96": ones96,
            "sel16": sel16, "sel96": sel96, "selred": selred,
            "mask": m, "maskb": (1.0 - m).copy(),
            "gam1": inputs["gamma1"].astype(np.float32)[:, None].copy(),
            "gam2": inputs["gamma2"].astype(np.float32)[:, None].copy(),
            "mwin": mwin, "mwout": mwout, "dwcol": dwcol,
            "dwdiag": dwdiag,
            "dmask2": np.ascontiguousarray(dmask2),
            "onesrow": np.ones((1, L), np.float32),
            "dmask128": dmask128,
            "qoff": qoff,
        })
    return maps


def kernel(**inputs) -> np.ndarray:
    from concourse.bass_utils import run_bass_kernel_spmd

    if "nc" not in _COMPILED:
        _COMPILED["nc"] = _build()
    nc = _COMPILED["nc"]
    maps = _host_inputs(inputs)
    res = run_bass_kernel_spmd(nc, maps, core_ids=list(range(8)))
    out = np.empty((2, C, H, W), np.float32)
    for c in range(8):
        b = c // 4
        q = c % 4
        out[b, :, 16 * q:16 * (q + 1), :] = \
            res.results[c]["out"].reshape(C, 16, W)
    return out

